# revision 14
# baseline (speedup 1.0000x reference)
"""ConvMultiHeadAttention Trainium2 kernel (8-core SPMD, batch+head sharded).

Module: conv1d(k=3,pad=1) Q/K proj, conv1d(k=1) V proj, 16-head attention
(head = channel%16), concat, linear out-proj.  B=2, S=2048, D=1024, d=64.

Sharding: each of the 8 cores owns 2 heads x both batches.  Conv weights are
row-sliced per core (128 output channels each, ordered [head0 d0..63,
head1 d0..63]); q/k/v inputs are replicated (conv contracts all 1024 input
channels).  Each core produces a y-partial [4096, 1024] = (its heads' attn
output) @ wc_slice^T; the host sums the 8 partials and adds the biases that
commute out (wc_b, and bv @ wc_slice^T since softmax weights sum to 1).

v2: all matmul operands bf16 (1 cyc/row vs 4 for fp32), conv-q pipelined
into the attention block loop so ACT (exp) overlaps conv PE work, esum via
bf16 DVE tree (4x mode), 8-PSUM-bank schedule:
  sps (scores, [128,2h,512]x2buf) 4 banks | vps (attn accum) 2 | mps (conv
  ps / r / rb / proj yp, 2buf) 2.

Per-core dataflow (all layouts partition-major):
  conv v    -> V_sb [s-chunk 128, 128ch] (x-chunk stationary, wv moving)
  conv k/q  -> kcT / qc_b [128ch, pos] bf16 (3 taps x 8 ci-tile matmuls,
               bias added on the ACT psum->sbuf copy)
  scoresT   = kcT_tile.T @ qc_b per 128-k chunk; heads h0/h1 run as
              concurrent row-group matmuls (stationary bases 0/64)
  E = exp(scores/8) bf16, one ACT op per chunk over both heads [128,2,512]
  attn outT accumulated over chunks: col-tiled head pair (0,0)/(0,64)
              into ONE psum bank (disjoint partition groups)
  esum      = tree-sum of the 16 E chunks on DVE (bf16, 4x mode)
  r = ones^T @ esum (PE, col-tiled pair); 1/r bf16; broadcast to 128
              partitions with K=1 ones matmuls; normalize fused into the
              mandatory psum->sbuf copy (tensor_mul)
  proj      y[s_chunk,1024] = outT_tile.T @ wcT, DMA [128,1024] f32
"""

import sys
import numpy as np
from contextlib import ExitStack

sys.path.insert(0, "/opt/trn_rl_repo")

import concourse.bass as bass
import concourse.tile as tile
from concourse import bacc, mybir
from concourse.bass_interp import get_hw_module
from concourse import bass2jax

F32 = mybir.dt.float32
BF16 = mybir.dt.bfloat16

NCORES = 8
B, S, D = 2, 2048, 1024
H, HD = 16, 64          # heads, head dim
CO = 128                # conv output channels per core (2 heads x 64)
SP = S + 2              # padded positions per batch for k=3 conv
NPOS = B * S            # 4096
NCHUNK = NPOS // 128    # 32 s-chunks


def build_module(repeat: int = 1, hw: bool = True):
    nc = bacc.Bacc("TRN2", target_bir_lowering=False, debug=False,
                   num_devices=NCORES)

    xq = nc.dram_tensor("xq", [D, B * SP], BF16, kind="ExternalInput").ap()
    xk = nc.dram_tensor("xk", [D, B * SP], BF16, kind="ExternalInput").ap()
    xv = nc.dram_tensor("xv", [D, NPOS], BF16, kind="ExternalInput").ap()
    wq = nc.dram_tensor("wq", [128, 24, 128], BF16, kind="ExternalInput").ap()
    wk = nc.dram_tensor("wk", [128, 24, 128], BF16, kind="ExternalInput").ap()
    wv = nc.dram_tensor("wv", [128, 8, 128], BF16, kind="ExternalInput").ap()
    wc = nc.dram_tensor("wc", [128, 1024], BF16, kind="ExternalInput").ap()
    bq = nc.dram_tensor("bq", [128, 1], F32, kind="ExternalInput").ap()
    bk = nc.dram_tensor("bk", [128, 1], F32, kind="ExternalInput").ap()
    y = nc.dram_tensor("y", [NPOS, D], F32, kind="ExternalOutput").ap()

    with tile.TileContext(nc) as tc, ExitStack() as ctx:
        wpool = ctx.enter_context(tc.tile_pool(name="wpool", bufs=1))
        cpool = ctx.enter_context(tc.tile_pool(name="cpool", bufs=1))
        xpool = ctx.enter_context(tc.tile_pool(name="xpool", bufs=2))
        epool = ctx.enter_context(tc.tile_pool(name="epool", bufs=1))
        spool = ctx.enter_context(tc.tile_pool(name="spool", bufs=1))

        # ---- persistent weights / consts ----
        wq_sb = wpool.tile([128, 24, 128], BF16)
        nc.sync.dma_start(wq_sb[:], wq[:])
        wk_sb = wpool.tile([128, 24, 128], BF16)
        wv_sb = wpool.tile([128, 8, 128], BF16)
        wc_sb = wpool.tile([128, 1024], BF16)
        bq_sb = wpool.tile([128, 1], F32)
        bk_sb = wpool.tile([128, 1], F32)
        nc.sync.dma_start(wk_sb[:], wk[:])
        nc.sync.dma_start(wv_sb[:], wv[:])
        nc.sync.dma_start(wc_sb[:], wc[:])
        nc.sync.dma_start(bq_sb[:], bq[:])
        nc.sync.dma_start(bk_sb[:], bk[:])
        onesr = wpool.tile([128, 2], BF16)
        nc.vector.memset(onesr[:], 1.0)
        ones1 = wpool.tile([33, 64], BF16)
        nc.vector.memset(ones1[:], 1.0)

        # ---- persistent activations ----
        kcT = cpool.tile([128, NPOS], BF16)
        V_sb = cpool.tile([128, NCHUNK, 128], BF16)
        qc_b = [cpool.tile([128, 512], BF16, tag=f"qc{i}", name=f"qc{i}")
                for i in range(8)]

        def body():
            with tc.tile_pool(name="pps", bufs=1, space="PSUM") as pps:
                # ====== conv k and conv v interleaved: each k-tile's PE
                # work (~7us) covers its own xk DMA plus two xv tiles ======
                def emit_convk(b, j):
                    col0 = b * SP + j * 512
                    xt = xpool.tile([128, 8, 514], BF16, tag="xqk",
                                    name="xtk")
                    nc.sync.dma_start(
                        xt[:],
                        xk[:, col0:col0 + 514].rearrange(
                            "(c p) i -> p c i", p=128))
                    ps = pps.tile([128, 512], F32, tag="mps", bufs=2,
                                  name="ckps")
                    n = 0
                    for t in range(3):
                        for c in range(8):
                            nc.tensor.matmul(
                                ps[:], wk_sb[:, t * 8 + c, :],
                                xt[:, c, t:t + 512],
                                start=(n == 0), stop=(n == 23))
                            n += 1
                    nc.scalar.activation(
                        kcT[:, b * S + j * 512: b * S + (j + 1) * 512],
                        ps[:], mybir.ActivationFunctionType.Identity,
                        bias=bk_sb[:, 0:1])

                def emit_convv(b, j):
                    col0 = b * S + j * 256
                    xt = xpool.tile([128, 8, 256], BF16, tag="xv", bufs=4,
                                    name="xtv")
                    nc.sync.dma_start(
                        xt[:],
                        xv[:, col0:col0 + 256].rearrange(
                            "(c p) i -> p c i", p=128))
                    for ch in range(2):
                        vp = pps.tile([128, 128], F32, tag="mps", bufs=2,
                                      name="vp")
                        for c in range(8):
                            nc.tensor.matmul(
                                vp[:], xt[:, c, ch * 128:(ch + 1) * 128],
                                wv_sb[:, c, :],
                                start=(c == 0), stop=(c == 7))
                        chunk = b * 16 + j * 2 + ch
                        nc.vector.tensor_copy(V_sb[:, chunk, :], vp[:])

                for b in range(B):
                    for j in range(4):
                        emit_convk(b, j)
                for b in range(B):
                    for j in range(8):
                        emit_convv(b, j)

                # ==== software-pipelined blocks: per iteration emit
                #   conv-q(b) [PE]  ->  tail(b-1) [DVE mul + proj + store]
                #   ->  head-rest(b) [scores/exp/attnV/esum/r-chain]
                # so the end-of-block dependency chain of b-1 resolves while
                # PE streams conv-q/scores of block b. ====

                def emit_convq(blk):
                    b, jq = divmod(blk, 4)
                    xt = xpool.tile([128, 8, 514], BF16, tag="xqk",
                                    name="xtq")
                    nc.sync.dma_start(
                        xt[:],
                        xq[:, b * SP + jq * 512: b * SP + jq * 512 + 514]
                        .rearrange("(c p) i -> p c i", p=128))
                    ps = pps.tile([128, 512], F32, tag="mps", bufs=2,
                                  name="cqps")
                    n = 0
                    for t in range(3):
                        for c in range(8):
                            nc.tensor.matmul(
                                ps[:], wq_sb[:, t * 8 + c, :],
                                xt[:, c, t:t + 512],
                                start=(n == 0), stop=(n == 23))
                            n += 1
                    return ps

                def emit_head(blk, cq_ps):
                    b, jq = divmod(blk, 4)
                    qc = qc_b[blk]
                    nc.scalar.activation(
                        qc[:], cq_ps[:],
                        mybir.ActivationFunctionType.Identity,
                        bias=bq_sb[:, 0:1])
                    # ---- scores^T + exp per 128-k chunk; esum tree
                    #      interleaved with the exp arrivals ----
                    e_tiles = []
                    l1 = l2 = None
                    l3 = []
                    for ik in range(16):
                        k0 = b * S + ik * 128
                        sps = pps.tile([128, 2, 512], F32, tag="sps",
                                       bufs=2, name="sps")
                        nc.tensor.matmul(
                            sps[:, 0, :], kcT[0:64, k0:k0 + 128],
                            qc[0:64, :], start=True, stop=True)
                        nc.tensor.matmul(
                            sps[:, 1, :], kcT[64:128, k0:k0 + 128],
                            qc[64:128, :], start=True, stop=True)
                        et = epool.tile([128, 2, 512], BF16,
                                        tag=f"e{ik}", bufs=2, name="et")
                        nc.scalar.activation(
                            et[:], sps[:],
                            mybir.ActivationFunctionType.Exp, scale=0.125)
                        e_tiles.append(et)
                        if ik % 2 == 1:
                            i = ik // 2
                            t1 = spool.tile([128, 2, 512], BF16,
                                            tag=f"t1{i % 2}", name="t1")
                            nc.vector.tensor_add(t1[:], e_tiles[ik - 1][:],
                                                 e_tiles[ik][:])
                            if i % 2 == 0:
                                l1 = t1
                            else:
                                t2 = spool.tile([128, 2, 512], BF16,
                                                tag=f"t2{(i // 2) % 2}",
                                                name="t2")
                                nc.vector.tensor_add(t2[:], l1[:], t1[:])
                                if (i // 2) % 2 == 0:
                                    l2 = t2
                                else:
                                    t3 = spool.tile([128, 2, 512], BF16,
                                                    tag=f"t3{i // 4}",
                                                    bufs=2, name="t3")
                                    nc.vector.tensor_add(t3[:], l2[:], t2[:])
                                    l3.append(t3)
                    # ---- attn @ V (col-tiled head pair, one bank) ----
                    vps = pps.tile([128, 512], F32, tag="vps", bufs=2,
                                   name="vps")
                    for ik in range(16):
                        chunk = b * 16 + ik
                        nc.tensor.matmul(
                            vps[0:64, :], V_sb[:, chunk, 0:64],
                            e_tiles[ik][:, 0, :],
                            start=(ik == 0), stop=(ik == 15),
                            tile_position=(0, 0),
                            skip_group_check=True)
                        nc.tensor.matmul(
                            vps[64:128, :], V_sb[:, chunk, 64:128],
                            e_tiles[ik][:, 1, :],
                            start=(ik == 0), stop=(ik == 15),
                            tile_position=(0, 64),
                            skip_group_check=True)
                    # ---- r = ones^T @ (l3_0 + l3_1), 1/r, broadcast ----
                    r_ps = pps.tile([33, 512], F32, tag="mps", bufs=2,
                                    name="rps")
                    for i in range(2):
                        nc.tensor.matmul(r_ps[0:1, :], onesr[:, 0:1],
                                         l3[i][:, 0, :], start=(i == 0),
                                         stop=(i == 1), tile_position=(0, 0),
                                         skip_group_check=True)
                        nc.tensor.matmul(r_ps[32:33, :], onesr[:, 1:2],
                                         l3[i][:, 1, :], start=(i == 0),
                                         stop=(i == 1), tile_position=(0, 32),
                                         skip_group_check=True)
                    rinv = spool.tile([33, 512], BF16, tag="rinv", bufs=2)
                    with nc.allow_low_precision(
                            reason="1/r in bf16; |rel err| ~4e-3 ok"):
                        nc.vector.reciprocal(rinv[0:1, :], r_ps[0:1, :])
                        nc.vector.reciprocal(rinv[32:33, :], r_ps[32:33, :])
                    rb_ps = pps.tile([128, 512], F32, tag="mps", bufs=2)
                    nc.tensor.matmul(rb_ps[0:64, :], ones1[0:1, :],
                                     rinv[0:1, :], start=True, stop=True,
                                     tile_position=(0, 0))
                    nc.tensor.matmul(rb_ps[64:128, :], ones1[32:33, :],
                                     rinv[32:33, :], start=True, stop=True,
                                     tile_position=(32, 64))
                    rb_sb = spool.tile([128, 512], BF16, tag="rbsb", bufs=2)
                    nc.vector.tensor_copy(rb_sb[:], rb_ps[:])
                    return (blk, vps, rb_sb)

                def emit_tail(st):
                    blk, vps, rb_sb = st
                    b, jq = divmod(blk, 4)
                    q0 = b * S + jq * 512
                    # normalize fused into the mandatory psum->sbuf copy
                    outT = spool.tile([128, 512], BF16, tag="outT", bufs=2)
                    nc.vector.tensor_mul(outT[:], vps[:], rb_sb[:])
                    for ch in range(4):
                        ysb = spool.tile([128, 2, 512], F32, tag="ysb",
                                         bufs=2)
                        for half in range(2):
                            yp = pps.tile([128, 512], F32, tag="mps",
                                          bufs=2)
                            nc.tensor.matmul(
                                yp[:], outT[:, ch * 128:(ch + 1) * 128],
                                wc_sb[:, half * 512:(half + 1) * 512],
                                start=True, stop=True)
                            nc.vector.tensor_copy(ysb[:, half, :], yp[:])
                        row0 = q0 + ch * 128
                        nc.sync.dma_start(y[row0:row0 + 128, :], ysb[:])

                state = None
                for blk in range(9):
                    cq = emit_convq(blk) if blk < 8 else None
                    if state is not None:
                        emit_tail(state)
                        state = None
                    if blk < 8:
                        state = emit_head(blk, cq)

        if repeat == 1:
            body()
        else:
            with tc.For_i(0, repeat, 1):
                body()

    nc.compile()
    if hw:
        nc.m = get_hw_module(nc.m)
    return nc


def host_prep(inputs):
    """Returns (in_maps, bias_y) — per-core input dicts + host-side bias."""
    import ml_dtypes
    q = np.asarray(inputs["q"], np.float32)
    k = np.asarray(inputs["k"], np.float32)
    v = np.asarray(inputs["v"], np.float32)
    wq_w = np.asarray(inputs["wq_w"], np.float32)
    wk_w = np.asarray(inputs["wk_w"], np.float32)
    wv_w = np.asarray(inputs["wv_w"], np.float32)
    wc_w = np.asarray(inputs["wc_w"], np.float32)

    def pad_T(x):  # [B,S,D] -> [D, B*(S+2)] zero-padded at batch edges
        out = np.zeros((D, B * SP), np.float32)
        xT = np.swapaxes(x, 1, 2)  # [B, D, S]
        for b in range(B):
            out[:, b * SP + 1: b * SP + 1 + S] = xT[b]
        return np.ascontiguousarray(out)

    xq = pad_T(q)
    xk = pad_T(k)
    xv = np.ascontiguousarray(
        np.swapaxes(v, 1, 2).transpose(1, 0, 2).reshape(D, NPOS))

    def pack_w3(w_dev):  # [128co, 1024ci, 3t] -> [p, (t c), m] = [128,24,128]
        a = w_dev.transpose(1, 2, 0)          # [ci, t, co]
        a = a.reshape(8, 128, 3, 128)         # [c, p, t, co]
        return np.ascontiguousarray(
            a.transpose(1, 2, 0, 3).reshape(128, 24, 128))

    def pack_w1(w_dev):  # [128co, 1024ci] -> [p, c, m] = [128, 8, 128]
        a = w_dev.T.reshape(8, 128, 128)      # [c, p, co]
        return np.ascontiguousarray(a.transpose(1, 0, 2))

    cast = lambda a: a.astype(ml_dtypes.bfloat16)
    in_maps = []
    bias_y = np.zeros((D,), np.float64)
    for dev in range(NCORES):
        heads = [2 * dev, 2 * dev + 1]
        rows = np.array([di * H + h for h in heads for di in range(HD)])
        feat = slice(2 * dev * HD, 2 * dev * HD + 128)
        wc_slice = np.ascontiguousarray(wc_w[:, feat].T)   # [128, 1024]
        bv_dev = np.asarray(inputs["wv_b"], np.float32)[rows]
        bias_y += bv_dev @ wc_slice
        in_maps.append({
            "xq": cast(xq), "xk": cast(xk), "xv": cast(xv),
            "wq": cast(pack_w3(wq_w[rows])),
            "wk": cast(pack_w3(wk_w[rows])),
            "wv": cast(pack_w1(wv_w[rows, :, 0])),
            "wc": cast(wc_slice),
            "bq": np.ascontiguousarray(
                np.asarray(inputs["wq_b"], np.float32)[rows][:, None]),
            "bk": np.ascontiguousarray(
                np.asarray(inputs["wk_b"], np.float32)[rows][:, None]),
        })
    bias_y += np.asarray(inputs["wc_b"], np.float32)
    return in_maps, bias_y.astype(np.float32)


class Runner:
    """Caches the compiled module + jitted SPMD callable (mirrors
    bass2jax.run_bass_via_pjrt, but reusable across calls)."""

    def __init__(self, repeat: int = 1, builder=None):
        import jax
        from jax.sharding import Mesh, PartitionSpec
        from jax.experimental.shard_map import shard_map
        from concourse.bass2jax import (
            _bass_exec_p, install_neuronx_cc_hook, partition_id_tensor)

        self.jax = jax
        nc = (builder or build_module)(repeat)
        self.nc = nc
        install_neuronx_cc_hook()
        assert nc.dbg_addr is None

        in_names, out_names, out_avals, zero_outs = [], [], [], []
        pname = nc.partition_id_tensor.name if nc.partition_id_tensor else None
        for alloc in nc.m.functions[0].allocations:
            if not isinstance(alloc, mybir.MemoryLocationSet):
                continue
            name = alloc.memorylocations[0].name
            if alloc.kind == "ExternalInput":
                if name != pname:
                    in_names.append(name)
            elif alloc.kind == "ExternalOutput":
                out_names.append(name)
                shape = tuple(alloc.tensor_shape)
                dt = mybir.dt.np(alloc.dtype)
                out_avals.append(jax.core.ShapedArray(shape, dt))
                zero_outs.append(np.zeros(shape, dt))
        self.in_names, self.out_names = in_names, out_names
        self.out_avals, self.zero_outs = out_avals, zero_outs
        n_params, n_outs = len(in_names), len(out_avals)
        all_names = in_names + out_names + ([pname] if pname else [])

        def _body(*args):
            operands = list(args)
            if pname:
                operands.append(partition_id_tensor())
            return tuple(_bass_exec_p.bind(
                *operands,
                out_avals=tuple(out_avals),
                in_names=tuple(all_names),
                out_names=tuple(out_names),
                lowering_input_output_aliases=(),
                sim_require_finite=True,
                sim_require_nnan=True,
                nc=nc))

        devices = jax.devices()[:NCORES]
        self.mesh = Mesh(np.asarray(devices), ("core",))
        self.sharded = jax.jit(
            shard_map(_body, mesh=self.mesh,
                      in_specs=(PartitionSpec("core"),) * (n_params + n_outs),
                      out_specs=(PartitionSpec("core"),) * n_outs,
                      check_rep=False),
            donate_argnums=tuple(range(n_params, n_params + n_outs)),
            keep_unused=True)

    def concat_inputs(self, in_maps):
        return [np.concatenate([np.asarray(m[n]) for m in in_maps], axis=0)
                for n in self.in_names]

    def concat_zeros(self):
        return [np.zeros((NCORES * z.shape[0], *z.shape[1:]), z.dtype)
                for z in self.zero_outs]

    def call(self, concat_in, concat_zero):
        """Returns device output arrays (not fetched)."""
        out = self.sharded(*concat_in, *concat_zero)
        self.jax.block_until_ready(out)
        return out

    def run(self, in_maps):
        out = self.call(self.concat_inputs(in_maps), self.concat_zeros())
        return [
            {n: np.asarray(out[i]).reshape(NCORES, *self.out_avals[i].shape)[c]
             for i, n in enumerate(self.out_names)}
            for c in range(NCORES)]


_CACHED = {}


def get_runner(repeat: int = 1) -> Runner:
    if repeat not in _CACHED:
        _CACHED[repeat] = Runner(repeat)
    return _CACHED[repeat]


def run(in_maps, repeat: int = 1):
    return get_runner(repeat).run(in_maps)


def kernel(**inputs) -> np.ndarray:
    in_maps, bias_y = host_prep(inputs)
    results = run(in_maps)
    y = np.zeros((NPOS, D), np.float64)
    for r in results:
        y += r["y"].astype(np.float64)
    y = y.astype(np.float32) + bias_y[None, :]
    return y.reshape(B, S, D)


if __name__ == "__main__":
    rng = np.random.default_rng(0)
    fake = {
        "q": rng.standard_normal((B, S, D)).astype(np.float32),
        "k": rng.standard_normal((B, S, D)).astype(np.float32),
        "v": rng.standard_normal((B, S, D)).astype(np.float32),
        "wq_w": (rng.standard_normal((D, D, 3)) / 32).astype(np.float32),
        "wq_b": np.zeros(D, np.float32),
        "wk_w": (rng.standard_normal((D, D, 3)) / 32).astype(np.float32),
        "wk_b": np.zeros(D, np.float32),
        "wv_w": (rng.standard_normal((D, D, 1)) / 32).astype(np.float32),
        "wv_b": np.zeros(D, np.float32),
        "wc_w": (rng.standard_normal((D, D)) / 32).astype(np.float32),
        "wc_b": np.zeros(D, np.float32),
    }
    out = kernel(**fake)
    print("kernel output", out.shape, out.dtype)
